# revision 18
# baseline (speedup 1.0000x reference)
"""Trainium2 Bass kernel for nn_DiscretisedBNF (discretised BNF loss).

Math reduction used on device: the reference's (B, D, K=128) clamped-CDF
bin sum collapses (Abel summation) to

    pO[b,d] = -127/256 + sum_{k=1..127} u_k * erf(z_k),
    z_k = (e_k - mu_x) * inv,   e_k = 2k/128 - 1,
    u_k = -1/128 (k<127),  u_127 = 125/256,
    inv = 1 / (sigma_x * sqrt(2))

verified exact vs the reference formula.

Sharding (8 cores, full inputs in, full output out):
  - mm1 (mu_cat @ W1) computed on every core (bf16, transposed layout
    hT = W1^T @ mu_cat^T so H lands on partitions),
  - W2 column-sharded: core i owns output columns {i*128..(i+1)*128-1}
    (mu_eps) and {1024+i*128..} (ln_sigma) -> mm2 is 1/8 per core,
  - binning data-parallel over the same d-slice: 32768 elements/core,
  - per-core output: 128 partial sums of sigma1^{-2t}*(x-pO)^2; host
    reduces and scales.

Binning pipeline per core: DVE computes inv and mu_x*inv, splits each
into exact bf16 (hi, lo) pairs; PE forms z tiles [128 edges, 1024 elems]
as a K=4 bf16 outer product (exact to ~2^-17); ACT runs one big Erf per
tile (PSUM -> SBUF bf16); PE contracts edges with the u-weights
(erf tile as stationary, [128,1] moving) giving q in PSUM [128, 256];
DVE computes sum of (sqw*(x + 127/256 - q))^2 per partition.
"""

import sys

sys.path.insert(0, "/opt/trn_rl_repo")

import numpy as np
import ml_dtypes

import concourse.bass as bass
import concourse.tile as tile
from concourse import bacc, mybir
from concourse.alu_op_type import AluOpType
from concourse.bass_utils import run_bass_kernel_spmd

B, D, H, K = 256, 1024, 2048, 128
NCORES = 8
DSL = D // NCORES  # 128 d-columns per core
SIGMA1 = 0.02
TMIN = 1e-10
LEAK = 0.01
C127 = 127.0 / 256.0

F32 = mybir.dt.float32
BF16 = mybir.dt.bfloat16
BFNP = ml_dtypes.bfloat16

N_GROUPS = 32          # binning groups per core
GELEMS = 1024          # elements per group (2 z-matmuls of N=512)
NELEMS = DSL * B       # 32768 elements per core


def _build(debug=False):
    nc = bacc.Bacc("TRN2", target_bir_lowering=False, debug=False,
                   num_devices=NCORES)

    d_xT = nc.dram_tensor("xT", (D, B), F32, kind="ExternalInput")
    d_nT = nc.dram_tensor("nT", (D, B), F32, kind="ExternalInput")
    d_xsl = nc.dram_tensor("x_sl", (64, 2 * B), F32, kind="ExternalInput")
    d_nsl = nc.dram_tensor("n_sl", (64, 2 * B), F32, kind="ExternalInput")
    d_w1 = nc.dram_tensor("w1", (D, H), BF16, kind="ExternalInput")
    d_w1r = nc.dram_tensor("w1row", (1, H), BF16, kind="ExternalInput")
    d_w2 = nc.dram_tensor("w2", (H, 2 * DSL), BF16, kind="ExternalInput")
    d_tv = nc.dram_tensor("tv", (1, B), BF16, kind="ExternalInput")
    d_b1r = nc.dram_tensor("b1r", (128, 16), F32, kind="ExternalInput")
    d_b2r = nc.dram_tensor("b2r", (64, 4), F32, kind="ExternalInput")
    d_bc = nc.dram_tensor("bc", (2 * 128, B), F32, kind="ExternalInput")
    d_bc64 = nc.dram_tensor("bc64", (4 * 64, 2 * B), F32, kind="ExternalInput")
    d_edg = nc.dram_tensor("edg", (4, 128), BF16, kind="ExternalInput")
    d_uv = nc.dram_tensor("uv", (128, 1), BF16, kind="ExternalInput")
    d_xqc = nc.dram_tensor("xqc", (128, B), F32, kind="ExternalInput")
    d_sqwq = nc.dram_tensor("sqwq", (128, B), F32, kind="ExternalInput")
    d_part = nc.dram_tensor("part", (128, 1), F32, kind="ExternalOutput")
    dbg = {}
    if debug:
        for nm, shp in [("dbg_me", (64, 2 * B)), ("dbg_ls", (64, 2 * B)),
                        ("dbg_inv", (64, 2 * B)), ("dbg_mx", (64, 2 * B)),
                        ("dbg_q", (128, B))]:
            dbg[nm] = nc.dram_tensor(nm, shp, F32, kind="ExternalOutput")

    MULT, ADD, SUB, BYP = (AluOpType.mult, AluOpType.add,
                           AluOpType.subtract, AluOpType.bypass)
    AF = mybir.ActivationFunctionType

    with tile.TileContext(nc) as tc:
        with (
            tc.tile_pool(name="consts", bufs=1) as cpool,
            tc.tile_pool(name="weights", bufs=1) as wpool,
            tc.tile_pool(name="work", bufs=1) as work,
            tc.tile_pool(name="stage", bufs=1) as stage,
        ):
            bc = []  # g1, g2 broadcast over 128 d-partitions (muT stage)
            for j in range(2):
                bt = cpool.tile([128, B], F32, tag=f"bc{j}")
                nc.sync.dma_start(bt[:], d_bc.ap()[j * 128:(j + 1) * 128, :])
                bc.append(bt)
            g1_bc, g2_bc = bc

            muT = work.tile([128, 8, B], BF16)
            hT = work.tile([128, 16, B], BF16)
            # prep stage runs in [64, 2, 256] layout (d = dh*64 + p) so the
            # flatten DMA gets 1KB-contiguous per-partition descriptors
            ME = work.tile([64, 2, B], F32)
            lnm = work.tile([64, 2, B], F32)
            w1 = wpool.tile([128, 8, H], BF16)
            w1r = wpool.tile([1, H], BF16)
            w2 = wpool.tile([128, 16, 2 * DSL], BF16)

            with (
                tc.tile_pool(name="xin", bufs=1) as xin,
                tc.tile_pool(name="psA", bufs=4,
                             space=bass.MemorySpace.PSUM) as psA,
            ):
                xt = xin.tile([128, 8, B], F32)
                nt = xin.tile([128, 8, B], F32)
                # interleave x/noise/W1 loads and muT compute per k-tile
                for k in range(8):
                    nc.sync.dma_start(xt[:, k, :], d_xT.ap()[k * 128:(k + 1) * 128, :])
                    nc.sync.dma_start(nt[:, k, :], d_nT.ap()[k * 128:(k + 1) * 128, :])
                    nc.sync.dma_start(w1[:, k, :], d_w1.ap()[k * 128:(k + 1) * 128, :])
                    t1 = xin.tile([128, B], F32, tag="mu_t1")
                    nc.vector.tensor_tensor(t1[:], xt[:, k, :], g1_bc[:], MULT)
                    t2 = xin.tile([128, B], F32, tag="mu_t2")
                    nc.vector.tensor_tensor(t2[:], nt[:, k, :], g2_bc[:], MULT)
                    nc.vector.tensor_tensor(muT[:, k, :], t1[:], t2[:], ADD)
                b1r = cpool.tile([128, 16], F32)
                nc.sync.dma_start(b1r[:], d_b1r.ap()[:])
                tvt = cpool.tile([1, B], BF16)
                nc.sync.dma_start(tvt[:], d_tv.ap()[:])
                nc.sync.dma_start(w1r[:], d_w1r.ap()[:])
                # binning-prep inputs (needed right after mm2)
                b2r = cpool.tile([64, 4], F32)
                nc.sync.dma_start(b2r[:], d_b2r.ap()[:])
                bc64 = []  # mf, Bv, rm, cexp in [64, 2, 256]
                for j in range(4):
                    bt = cpool.tile([64, 2, B], F32, tag=f"bc64_{j}")
                    nc.sync.dma_start(bt[:], d_bc64.ap()[j * 64:(j + 1) * 64, :])
                    bc64.append(bt)
                mf_bc, bv_bc, rm_bc, ce_bc = bc64
                xsl = work.tile([64, 2, B], F32)
                nc.sync.dma_start(xsl[:], d_xsl.ap()[:])
                nsl = work.tile([64, 2, B], F32)
                nc.sync.dma_start(nsl[:], d_nsl.ap()[:])
                # mu_x partials that do not depend on mm2 run during mm1
                a1 = work.tile([64, 2, B], F32)
                nc.vector.tensor_tensor(a1[:], xsl[:], mf_bc[:], MULT)
                a2 = work.tile([64, 2, B], F32)
                nc.vector.tensor_tensor(a2[:], nsl[:], bv_bc[:], MULT)
                s = work.tile([64, 2, B], F32)
                nc.vector.tensor_tensor(s[:], a1[:], a2[:], ADD)
                for k in range(16):
                    nc.sync.dma_start(w2[:, k, :], d_w2.ap()[k * 128:(k + 1) * 128, :])
                edg = cpool.tile([4, 128], BF16)
                nc.sync.dma_start(edg[:], d_edg.ap()[:])
                uv = cpool.tile([128, 1], BF16)
                nc.sync.dma_start(uv[:], d_uv.ap()[:])

                # mm1: hT[m] = LeakyReLU(W1^T mu_cat^T + b1)  (16 M-tiles)
                for m in range(16):
                    ph = psA.tile([128, B], F32, tag="ph")
                    ms = slice(m * 128, (m + 1) * 128)
                    for k in range(8):
                        nc.tensor.matmul(ph[:], w1[:, k, ms], muT[:, k, :],
                                         start=(k == 0), stop=False)
                    nc.tensor.matmul(ph[:], w1r[:, ms], tvt[:],
                                     start=False, stop=True)
                    nc.scalar.activation(hT[:, m, :], ph[:], AF.Lrelu,
                                         bias=b1r[:, m:m + 1], scale=1.0,
                                         alpha=LEAK)

                # mm2: out^T in 4 M-tiles of 64 rows; ln_sigma halves first
                # so the exp/inv chain starts as early as possible
                po_map = {}
                for mo in (2, 3, 0, 1):
                    po = psA.tile([64, B], F32, tag="po")
                    mos = slice(mo * 64, (mo + 1) * 64)
                    for k in range(16):
                        nc.tensor.matmul(po[:], w2[:, k, mos], hT[:, k, :],
                                         start=(k == 0), stop=(k == 15))
                    if mo < 2:
                        nc.vector.tensor_scalar_add(ME[:, mo, :], po[:],
                                                    b2r[:, mo:mo + 1])
                    else:
                        nc.vector.scalar_tensor_tensor(
                            lnm[:, mo - 2, :], po[:], b2r[:, mo:mo + 1],
                            mf_bc[:, mo - 2, :], op0=ADD, op1=MULT)

            # ---- binning prep ([64, 2, 256]); chain A (inv) first so the
            # ih/il flatten DMAs launch while chain B (mu_x) still runs
            QT = stage.tile([64, 4, 2, B], BF16)
            R = stage.tile([4, NELEMS], BF16)
            flat_engines = [nc.sync, nc.scalar, nc.gpsimd]
            HALF = NELEMS // 2
            dep_srcs = []

            def flatten_row(r):
                for hh in range(2):
                    eng = flat_engines[(2 * r + hh) % 3]
                    eng.dma_start(
                        R[r:r + 1, hh * HALF:(hh + 1) * HALF],
                        QT[hh * 32:(hh + 1) * 32, r, :, :])

            ei = work.tile([64, 2, B], F32)
            dep_srcs.append(nc.scalar.activation(ei[:], lnm[:], AF.Exp,
                                                 bias=0.0, scale=-1.0))
            inv = work.tile([64, 2, B], F32)
            dep_srcs.append(nc.vector.tensor_tensor(inv[:], ei[:], ce_bc[:], MULT))
            dep_srcs.append(nc.vector.tensor_copy(QT[:, 0, :, :], inv[:]))  # ih
            ihf = work.tile([64, 2, B], F32)
            nc.vector.tensor_copy(ihf[:], QT[:, 0, :, :])
            flatten_row(0)
            dep_srcs.append(
                nc.vector.tensor_tensor(QT[:, 1, :, :], inv[:], ihf[:], SUB))  # il
            flatten_row(1)
            a4 = work.tile([64, 2, B], F32)
            dep_srcs.append(nc.vector.tensor_tensor(a4[:], rm_bc[:], ME[:], MULT))
            mu_x = work.tile([64, 2, B], F32)
            dep_srcs.append(nc.vector.tensor_tensor(mu_x[:], s[:], a4[:], SUB))
            mx = work.tile([64, 2, B], F32)
            dep_srcs.append(nc.vector.tensor_tensor(mx[:], mu_x[:], inv[:], MULT))
            dep_srcs.append(nc.vector.tensor_copy(QT[:, 2, :, :], mx[:]))   # hi
            hif = work.tile([64, 2, B], F32)
            nc.vector.tensor_copy(hif[:], QT[:, 2, :, :])
            flatten_row(2)
            dep_srcs.append(
                nc.vector.tensor_tensor(QT[:, 3, :, :], mx[:], hif[:], SUB))  # lo
            flatten_row(3)

            if debug:
                for nm, src in [("dbg_me", ME), ("dbg_ls", lnm),
                                ("dbg_inv", inv), ("dbg_mx", mx)]:
                    nc.sync.dma_start(dbg[nm].ap()[:], src[:])

            # ---- binning main loop -------------------------------------
            with (
                tc.tile_pool(name="psZ", bufs=2,
                             space=bass.MemorySpace.PSUM) as psZ,
                tc.tile_pool(name="psW", bufs=1,
                             space=bass.MemorySpace.PSUM) as psW,
                tc.tile_pool(name="psQ", bufs=1,
                             space=bass.MemorySpace.PSUM) as psQ,
                tc.tile_pool(name="erf", bufs=2) as epool,
            ):
                # keep the PE HAM warm through the prep/flatten window:
                # dummy matmuls dep-chained onto the prep ops
                warm = psW.tile([128, B], F32)
                for wi, src_i in enumerate(dep_srcs):
                    wm = nc.tensor.matmul(warm[:], w1[:, 0, 0:128],
                                          muT[:, 0, :], start=True, stop=True)
                    tile.add_dep_helper(src_i.ins, wm.ins,
                                        reason="pe-ham-warmer")

                # groups of 1536 elements (3 z-matmuls, one erf) + 512 tail
                sizes = [1536] * 21 + [512]
                q = psQ.tile([128, B], F32)
                base = 0
                for g, gel in enumerate(sizes):
                    zt = psZ.tile([128, 1536], F32, tag="zt")
                    nz = gel // 512
                    for h in range(nz):
                        nc.tensor.matmul(
                            zt[:, h * 512:(h + 1) * 512], edg[:],
                            R[:, base + h * 512: base + (h + 1) * 512],
                            start=True, stop=True)
                    et = epool.tile([128, 1536], BF16, tag="et")
                    nc.scalar.activation(et[:, 0:gel], zt[:, 0:gel], AF.Erf,
                                         bias=0.0, scale=1.0)
                    for j in range(gel // 128):
                        c = base // 128 + j
                        nc.tensor.matmul(q[:, c:c + 1],
                                         et[:, j * 128:(j + 1) * 128], uv[:],
                                         start=True, stop=True)
                    base += gel

                # tail: part = sum_cols (sqw*(xqc - q))^2
                xqc = cpool.tile([128, B], F32)
                nc.sync.dma_start(xqc[:], d_xqc.ap()[:])
                sqwq = cpool.tile([128, B], F32)
                nc.sync.dma_start(sqwq[:], d_sqwq.ap()[:])
                e1 = work.tile([128, B], F32)
                nc.vector.scalar_tensor_tensor(e1[:], q[:], -1.0, xqc[:],
                                               op0=MULT, op1=ADD)
                dw = work.tile([128, B], F32)
                nc.vector.tensor_tensor(dw[:], e1[:], sqwq[:], MULT)
                dw2 = work.tile([128, B], F32)
                part = work.tile([128, 1], F32)
                nc.vector.scalar_tensor_tensor(dw2[:], dw[:], 1.0, dw[:],
                                               op0=BYP, op1=MULT,
                                               accum_out=part[:])
                nc.sync.dma_start(d_part.ap()[:], part[:])
                if debug:
                    qsb = work.tile([128, B], F32)
                    nc.vector.tensor_copy(qsb[:], q[:])
                    nc.sync.dma_start(dbg["dbg_q"].ap()[:], qsb[:])

    nc.compile()
    return nc


def host_prep(x, t, noise, W1, b1, W2, b2):
    """Build the per-core in_maps (host-side sharding + tiny per-row math)."""
    f32 = np.float32
    tv = t[:, 0].astype(f32)
    gamma = (1.0 - np.power(f32(SIGMA1), f32(2.0) * tv)).astype(f32)
    low = tv < TMIN
    mf = np.where(low, f32(0.0), f32(1.0)).astype(f32)
    gsafe = np.where(gamma > 0, gamma, f32(1.0)).astype(f32)
    r = np.sqrt((1.0 - gsafe) / gsafe).astype(f32)
    rsafe = np.where(r > 0, r, f32(1.0)).astype(f32)
    g1 = gamma
    g2 = (gamma * (1.0 - gamma)).astype(f32)
    bv = ((1.0 - gamma) * mf).astype(f32)
    rm = (r * mf).astype(f32)
    cexp = np.where(low, f32(1.0 / np.sqrt(2.0)),
                    (1.0 / (rsafe * np.sqrt(2.0))).astype(f32)).astype(f32)
    sqw = np.power(f32(SIGMA1), -tv).astype(f32)

    bc = np.concatenate([np.broadcast_to(v, (128, B))
                         for v in (g1, g2)], axis=0)
    bc = np.ascontiguousarray(bc, dtype=f32)
    bc64 = np.concatenate([np.broadcast_to(np.tile(v, 2), (64, 2 * B))
                           for v in (mf, bv, rm, cexp)], axis=0)
    bc64 = np.ascontiguousarray(bc64, dtype=f32)

    def to64(a128):
        # [128 d, 256 b] -> [64 p, 512] with [p, dh*256+b] = a[dh*64+p, b]
        return np.ascontiguousarray(
            a128.reshape(2, 64, B).transpose(1, 0, 2).reshape(64, 2 * B))

    e = (2.0 * np.arange(1, K) / K - 1.0).astype(f32)  # 127 edges
    edg = np.zeros((4, 128), dtype=BFNP)
    edg[0, :127] = e.astype(BFNP)
    edg[1, :127] = e.astype(BFNP)
    edg[2, :127] = BFNP(-1.0)
    edg[3, :127] = BFNP(-1.0)
    uvec = np.zeros((128, 1), dtype=BFNP)
    uvec[:126, 0] = BFNP(-1.0 / K)
    uvec[126, 0] = BFNP(125.0 / 256.0)

    xT = np.ascontiguousarray(x.T, dtype=f32)
    nT = np.ascontiguousarray(noise.T, dtype=f32)
    w1b = np.ascontiguousarray(W1[:D].astype(BFNP))
    w1rb = np.ascontiguousarray(W1[D:D + 1].astype(BFNP))
    tvb = np.ascontiguousarray(tv.astype(BFNP).reshape(1, B))
    b1r = np.ascontiguousarray(b1.reshape(16, 128).T, dtype=f32)

    # q layout index math: flat = col*128 + p ;
    # flat = p64*512 + dh*256 + b with d_local = dh*64 + p64
    p_idx = np.arange(128)[:, None]
    c_idx = np.arange(B)[None, :]
    flat = c_idx * 128 + p_idx
    d_l = (flat % 512) // B * 64 + flat // 512
    b_i = flat % B
    sqwq = np.ascontiguousarray(sqw[b_i], dtype=f32)

    in_maps = []
    for i in range(NCORES):
        cols = np.concatenate([np.arange(i * DSL, (i + 1) * DSL),
                               1024 + np.arange(i * DSL, (i + 1) * DSL)])
        w2b = np.ascontiguousarray(W2[:, cols].astype(BFNP))
        b2sl = b2[cols].astype(f32)
        b2r = np.ascontiguousarray(b2sl.reshape(4, 64).T, dtype=f32)
        xqc = np.ascontiguousarray(
            x[b_i, i * DSL + d_l].astype(f32) + f32(C127), dtype=f32)
        in_maps.append({
            "xT": xT, "nT": nT,
            "x_sl": to64(xT[i * DSL:(i + 1) * DSL]),
            "n_sl": to64(nT[i * DSL:(i + 1) * DSL]),
            "w1": w1b, "w1row": w1rb, "w2": w2b, "tv": tvb,
            "b1r": b1r, "b2r": b2r, "bc": bc, "bc64": bc64,
            "edg": edg, "uv": uvec, "xqc": xqc, "sqwq": sqwq,
        })
    return in_maps


_nc_cache = {}


def get_nc(debug=False):
    if debug not in _nc_cache:
        _nc_cache[debug] = _build(debug)
    return _nc_cache[debug]


def run_on_cores(inputs, trace=False, debug=False, tmpdir=None):
    nc = get_nc(debug)
    in_maps = host_prep(**inputs)
    res = run_bass_kernel_spmd(nc, in_maps, core_ids=list(range(NCORES)),
                               trace=trace, tmpdir=tmpdir)
    total = np.float32(0.0)
    for i in range(NCORES):
        total += res.results[i]["part"].astype(np.float32).sum()
    loss = np.float32(-np.log(np.float32(SIGMA1)) * total / np.float32(B * D))
    return loss, res


def kernel(**inputs):
    loss, _ = run_on_cores(inputs)
    return np.asarray(loss, dtype=np.float32)
